# revision 14
# baseline (speedup 1.0000x reference)
"""Single-head attention (B=4, S=2048, E=1024, H=64) on 8 TRN2 NeuronCores.

Sharding: each batch b is handled by a core pair; core 2b takes keys/values
[0:1024), core 2b+1 takes [1024:2048). Each core computes, for ALL 2048
queries of its batch, the unnormalized attention numerator and denominator
over its key half; the host sums the two halves and divides.

Precision: matmul inputs are fp16 (1 cycle/row on the PE like bf16, but
8x finer mantissa -- all tensors here are unit-scale Gaussians, well inside
fp16 range). Only the exp outputs (up to e^50) need bf16's exponent range:
expT / v / ones are bf16. PSUM accumulation is fp32 and the final softmax
combine is fp32 on the host.

Measured v2 lessons baked in:
  - DMA packets = per-partition contiguous lines; 1-2KB lines run ~180GB/s
    vs ~310GB/s at 8KB+. Host pre-packs x partition-major so every big DMA
    has 8-16KB lines, and there are only ~14 DMAs total (each dma_start
    costs ~620ns of issue time on its queue engine).
  - The PE HAM clock gate re-throttles after ~3.4us idle; a trickle of
    N=128 dummy matmuls covers the initial DMA fill so the first real
    projection runs at 2.4GHz.
  - ACT does exp ONLY (~690ns per 512-wide chunk incl overhead); the exp
    table set is pre-loaded at t=0 (one-time ~1.3us DMA) under the input
    DMA. All PSUM->SBUF copies go to DVE.

DMA priority order: w | k-x | q-x blocks 0,1 | v-x | q-x blocks 2,3, so
scoresT -> exp -> AV chase the q stream and the tail after the last q byte
is ~proj + 8 exp chunks + AV + out.
"""

import numpy as np

_B, _S, _E, _H = 4, 2048, 1024, 64
_P = 128
_EC = _E // _P          # 8 E-chunks
_SK = _S // 2           # 1024 keys per core
_SKC = _SK // _P        # 8 sk chunks
_NQ = _S // 512         # 4 query 512-blocks
_NWU = 48               # PE keep-warm dummy matmuls

_built = None


def _build():
    import concourse.bacc as bacc
    import concourse.mybir as mybir
    import concourse.tile as tile

    f32 = mybir.dt.float32
    f16 = mybir.dt.float16
    bf16 = mybir.dt.bfloat16
    Exp = mybir.ActivationFunctionType.Exp

    nc = bacc.Bacc("TRN2", target_bir_lowering=False, debug=False,
                   enable_asserts=False, num_devices=8)

    xk_d = nc.dram_tensor("xk", [_P, _EC, _SK], f16, kind="ExternalInput")
    xq_d = nc.dram_tensor("xq", [_NQ, _P, _EC, 512], f16, kind="ExternalInput")
    xv_d = nc.dram_tensor("xv", [_P, _EC, _SK], f16, kind="ExternalInput")
    w_d = nc.dram_tensor("w", [_P, 3, _EC, _H], f16, kind="ExternalInput")
    bs_d = nc.dram_tensor("bs", [_H, 3], f32, kind="ExternalInput")
    id_d = nc.dram_tensor("ident", [_H, _H], f16, kind="ExternalInput")
    out_d = nc.dram_tensor("out", [_H + 1, _S], f32, kind="ExternalOutput")

    with tile.TileContext(nc) as tc:
        with (
            tc.tile_pool(name="persist", bufs=1) as persist,
        ):
            w_sb = persist.tile([_P, 3, _EC, _H], f16)
            bs_sb = persist.tile([_H, 3], f32)
            id_sb = persist.tile([_H, _H], f16)

            xk_sb = persist.tile([_P, _EC, _SK], f16)
            xq_sb = [persist.tile([_P, _EC, 512], f16, name=f"xq{j}")
                     for j in range(_NQ)]
            xv_sb = persist.tile([_P, _EC, _SK], f16)

            kqT_sb = persist.tile([_H, _SK + _S], f16)   # kT | qT
            vT_sb = persist.tile([_H, _SK], f16)
            v_sb = persist.tile([_P, _SKC, _H + 1], bf16)  # v with ones col
            expT = persist.tile([_P, _SKC, _S], bf16)
            oT_sb = persist.tile([_H + 1, _S], f32)

            wu_sb = persist.tile([_P, 512], f16)
            wu_act = persist.tile([_P, 32], bf16)
            nc.vector.memset(wu_sb[:], 0.0)

            nc.vector.memset(v_sb[:, :, _H:_H + 1], 1.0)

            # x DMAs split across the sync and scalar queues -- the one
            # configuration measured to deliver ~2x single-queue aggregate
            # (~370GB/s; gpsimd participation degrades it). The scalar
            # queue's dma_start enqueues retire by ~18us and the exp stream
            # behind them on the in-order ACT queue starts right after,
            # which is the pipeline floor anyway (k+q0 + proj). Priority on
            # both queues: w | k | q0 | q1 | v | q2 | q3.
            nc.scalar.dma_start(w_sb[:], w_d.ap())
            nc.scalar.dma_start(bs_sb[:], bs_d.ap())
            nc.scalar.dma_start(xk_sb[:, 4:8], xk_d.ap()[:, 4:8])
            nc.sync.dma_start(xk_sb[:, 0:4], xk_d.ap()[:, 0:4])
            nc.scalar.dma_start(xq_sb[0][:, 4:8], xq_d.ap()[0, :, 4:8])
            nc.sync.dma_start(xq_sb[0][:, 0:4], xq_d.ap()[0, :, 0:4])
            nc.scalar.dma_start(xq_sb[1][:, 4:8], xq_d.ap()[1, :, 4:8])
            nc.sync.dma_start(xq_sb[1][:, 0:4], xq_d.ap()[1, :, 0:4])
            nc.scalar.dma_start(xv_sb[:, 4:8], xv_d.ap()[:, 4:8])
            nc.sync.dma_start(xv_sb[:, 0:4], xv_d.ap()[:, 0:4])
            nc.scalar.dma_start(xq_sb[2][:, 4:8], xq_d.ap()[2, :, 4:8])
            nc.sync.dma_start(xq_sb[2][:, 0:4], xq_d.ap()[2, :, 0:4])
            nc.scalar.dma_start(xq_sb[3][:, 4:8], xq_d.ap()[3, :, 4:8])
            nc.sync.dma_start(xq_sb[3][:, 0:4], xq_d.ap()[3, :, 0:4])
            nc.sync.dma_start(id_sb[:], id_d.ap())

            with tc.tile_pool(name="ps", bufs=1, space="PSUM") as ps_pool:
                # PE warmup burst + keep-warm trickle (HAM clock gate) on
                # memset data; spans the initial x DMA fill
                wu_ps = ps_pool.tile([_P, 512], f32, name="wu", tag="pj",
                                     bufs=2)
                nc.tensor.matmul(wu_ps[:_H, :], wu_sb[:, :_H], wu_sb[:],
                                 start=True, stop=True,
                                 skip_group_check=True)
                for _ in range(_NWU):
                    nc.tensor.matmul(wu_ps[:_H, :_P], wu_sb[:, :_H],
                                     wu_sb[:, :_P], start=True, stop=True,
                                     skip_group_check=True)
                # ACT exp table-set preload (~1.3us), hides under input DMA
                nc.scalar.activation(wu_act[:], wu_sb[:, :32], Exp)

                def proj(widx, rhs_of_e, dst, bias_col):
                    p = ps_pool.tile([_H, 512], f32, name="pj", tag="pj",
                                     bufs=2, padded_shape=[_P, 512])
                    for e in range(_EC):
                        nc.tensor.matmul(
                            p[:], w_sb[:, widx, e, :], rhs_of_e(e),
                            start=(e == 0), stop=(e == _EC - 1),
                        )
                    nc.vector.tensor_scalar_add(dst, p[:], bias_col)

                def scores_exp(j):
                    # two sk-chunks' scores land in one 2-bank PSUM tile so
                    # a single ACTIVATE (the ~270ns/instr overhead engine)
                    # exps 1024 elems/lane at once; dst is a strided
                    # [P, 2, 512] slice of expT
                    rhs = kqT_sb[:, _SK + j * 512:_SK + (j + 1) * 512]
                    for si in range(_SKC // 2):
                        sc = ps_pool.tile([_P, 1024], f32, name="sc", tag="sc",
                                          bufs=2)
                        nc.tensor.matmul(
                            sc[:, 0:512],
                            kqT_sb[:, (2 * si) * _P:(2 * si + 1) * _P],
                            rhs, start=True, stop=True,
                            skip_group_check=True,
                        )
                        nc.tensor.matmul(
                            sc[:, 512:1024],
                            kqT_sb[:, (2 * si + 1) * _P:(2 * si + 2) * _P],
                            rhs, start=True, stop=True,
                            skip_group_check=True,
                        )
                        nc.scalar.activation(
                            expT[:, 2 * si:2 * si + 2, j * 512:(j + 1) * 512],
                            sc[:], Exp)

                def av(j):
                    po = ps_pool.tile([_H + 1, 512], f32, name="po", tag="av",
                                      bufs=2, padded_shape=[_P, 512])
                    for skc in range(_SKC):
                        nc.tensor.matmul(
                            po[:], v_sb[:, skc, :],
                            expT[:, skc, j * 512:(j + 1) * 512],
                            start=(skc == 0), stop=(skc == _SKC - 1),
                        )
                    nc.vector.tensor_copy(oT_sb[:, j * 512:(j + 1) * 512],
                                          po[:])
                    eng = nc.scalar if j == 3 else nc.sync
                    eng.dma_start(out_d.ap()[:, j * 512:(j + 1) * 512],
                                  oT_sb[:, j * 512:(j + 1) * 512])

                bk = bs_sb[:, 0:1]
                bq = bs_sb[:, 1:2]
                bv = bs_sb[:, 2:3]

                # k projections, then q blocks 0/1 with scores+exp chasing
                # the DMA stream
                for kb in range(2):
                    proj(0, lambda e, kb=kb: xk_sb[:, e, kb * 512:(kb + 1) * 512],
                         kqT_sb[:, kb * 512:(kb + 1) * 512], bk)
                for j in (0, 1):
                    proj(1, lambda e, j=j: xq_sb[j][:, e, :],
                         kqT_sb[:, _SK + j * 512:_SK + (j + 1) * 512], bq)
                    scores_exp(j)

                # v projections + PE-transpose rebuild of v (ones col DMA'd)
                for vb in range(2):
                    proj(2, lambda e, vb=vb: xv_sb[:, e, vb * 512:(vb + 1) * 512],
                         vT_sb[:, vb * 512:(vb + 1) * 512], bv)
                for skc in range(_SKC):
                    pvt = ps_pool.tile([_P, _H], f16, name="pvt", tag="sc",
                                       bufs=2, padded_shape=[_P, 1024])
                    nc.tensor.transpose(
                        pvt[:], vT_sb[:, skc * _P:(skc + 1) * _P], id_sb[:])
                    nc.vector.tensor_copy(v_sb[:, skc, :_H], pvt[:])

                av(0)
                av(1)
                for j in (2, 3):
                    proj(1, lambda e, j=j: xq_sb[j][:, e, :],
                         kqT_sb[:, _SK + j * 512:_SK + (j + 1) * 512], bq)
                    scores_exp(j)
                av(2)
                av(3)

    nc.compile()
    return nc


def _prep_core(query, key, value, Wq, bq, Wk, bk, Wv, bv, core):
    b, half = core // 2, core % 2
    xkT = key[b].T[:, half * _SK:(half + 1) * _SK]         # [E, SK]
    xqT = query[b].T                                       # [E, S]
    xvT = value[b].T[:, half * _SK:(half + 1) * _SK]
    xk = xkT.reshape(_EC, _P, _SK).transpose(1, 0, 2)      # [P, EC, SK]
    xq = xqT.reshape(_EC, _P, _NQ, 512).transpose(2, 1, 0, 3)
    xv = xvT.reshape(_EC, _P, _SK).transpose(1, 0, 2)
    w = np.stack(
        [Wk.reshape(_EC, _P, _H), Wq.reshape(_EC, _P, _H),
         Wv.reshape(_EC, _P, _H)], axis=0,
    ).transpose(2, 0, 1, 3)                                # [P, 3, EC, H]
    bs = np.stack(
        [np.asarray(bk, dtype=np.float32).ravel(),
         np.asarray(bq, dtype=np.float32).ravel(),
         np.asarray(bv, dtype=np.float32).ravel()], axis=1,
    )                                                      # [H, 3]
    return {
        "xk": np.ascontiguousarray(xk).astype(np.float16),
        "xq": np.ascontiguousarray(xq).astype(np.float16),
        "xv": np.ascontiguousarray(xv).astype(np.float16),
        "w": np.ascontiguousarray(w).astype(np.float16),
        "bs": np.ascontiguousarray(bs, dtype=np.float32),
        "ident": np.eye(_H, dtype=np.float16),
    }


def _get_built():
    global _built
    if _built is None:
        _built = _build()
    return _built


def kernel(query, key, value, Wq, bq, Wk, bk, Wv, bv, _trace=False):
    from concourse.bass_utils import run_bass_kernel_spmd

    query = np.asarray(query, dtype=np.float32)
    key = np.asarray(key, dtype=np.float32)
    value = np.asarray(value, dtype=np.float32)
    Wq = np.asarray(Wq, dtype=np.float32)
    Wk = np.asarray(Wk, dtype=np.float32)
    Wv = np.asarray(Wv, dtype=np.float32)

    nc = _get_built()
    in_maps = [
        _prep_core(query, key, value, Wq, bq, Wk, bk, Wv, bv, c) for c in range(8)
    ]
    res = run_bass_kernel_spmd(nc, in_maps, core_ids=list(range(8)), trace=_trace)
    out = np.empty((_B, _S, _H), dtype=np.float32)
    for b in range(_B):
        oA = res.results[2 * b]["out"]      # [H+1, S]
        oB = res.results[2 * b + 1]["out"]
        num = oA[: _H] + oB[: _H]
        den = oA[_H] + oB[_H]
        out[b] = (num / den).T
    if _trace:
        kernel.last_result = res
    return out


# revision 15
# speedup vs baseline: 1.2674x; 1.2674x over previous
"""Single-head attention (B=4, S=2048, E=1024, H=64) on 8 TRN2 NeuronCores.

Sharding: each batch b is handled by a core pair; core 2b takes keys/values
[0:1024), core 2b+1 takes [1024:2048). Each core computes, for ALL 2048
queries of its batch, the unnormalized attention numerator and denominator
over its key half; the host sums the two halves and divides.

Precision: matmul inputs are fp16 (1 cycle/row on the PE like bf16, but
8x finer mantissa -- all tensors here are unit-scale Gaussians, well inside
fp16 range). Only the exp outputs (up to e^50) need bf16's exponent range:
expT / v / ones are bf16. PSUM accumulation is fp32 and the final softmax
combine is fp32 on the host.

Measured v2 lessons baked in:
  - DMA packets = per-partition contiguous lines; 1-2KB lines run ~180GB/s
    vs ~310GB/s at 8KB+. Host pre-packs x partition-major so every big DMA
    has 8-16KB lines, and there are only ~14 DMAs total (each dma_start
    costs ~620ns of issue time on its queue engine).
  - The PE HAM clock gate re-throttles after ~3.4us idle; a trickle of
    N=128 dummy matmuls covers the initial DMA fill so the first real
    projection runs at 2.4GHz.
  - ACT does exp ONLY (~690ns per 512-wide chunk incl overhead); the exp
    table set is pre-loaded at t=0 (one-time ~1.3us DMA) under the input
    DMA. All PSUM->SBUF copies go to DVE.

DMA priority order: w | k-x | q-x blocks 0,1 | v-x | q-x blocks 2,3, so
scoresT -> exp -> AV chase the q stream and the tail after the last q byte
is ~proj + 8 exp chunks + AV + out.
"""

import numpy as np

_B, _S, _E, _H = 4, 2048, 1024, 64
_P = 128
_EC = _E // _P          # 8 E-chunks
_SK = _S // 2           # 1024 keys per core
_SKC = _SK // _P        # 8 sk chunks
_NQ = _S // 512         # 4 query 512-blocks
_NWU = 48               # PE keep-warm dummy matmuls

_built = None


def _build():
    import concourse.bacc as bacc
    import concourse.mybir as mybir
    import concourse.tile as tile

    f32 = mybir.dt.float32
    f16 = mybir.dt.float16
    bf16 = mybir.dt.bfloat16
    Exp = mybir.ActivationFunctionType.Exp

    nc = bacc.Bacc("TRN2", target_bir_lowering=False, debug=False,
                   enable_asserts=False, num_devices=8)

    xk_d = nc.dram_tensor("xk", [_P, _EC, _SK], f16, kind="ExternalInput")
    xq_d = nc.dram_tensor("xq", [_NQ, _P, _EC, 512], f16, kind="ExternalInput")
    xv_d = nc.dram_tensor("xv", [_P, _EC, _SK], f16, kind="ExternalInput")
    w_d = nc.dram_tensor("w", [_P, 3, _EC, _H], f16, kind="ExternalInput")
    bs_d = nc.dram_tensor("bs", [_H, 3], f32, kind="ExternalInput")
    id_d = nc.dram_tensor("ident", [_H, _H], f16, kind="ExternalInput")
    out_d = nc.dram_tensor("out", [_H + 1, _S], f32, kind="ExternalOutput")

    with tile.TileContext(nc) as tc:
        with (
            tc.tile_pool(name="persist", bufs=1) as persist,
        ):
            w_sb = persist.tile([_P, 3, _EC, _H], f16)
            bs_sb = persist.tile([_H, 3], f32)
            id_sb = persist.tile([_H, _H], f16)

            xk_sb = persist.tile([_P, _EC, _SK], f16)
            xq_sb = [persist.tile([_P, _EC, 512], f16, name=f"xq{j}")
                     for j in range(_NQ)]
            xv_sb = persist.tile([_P, _EC, _SK], f16)

            kqT_sb = persist.tile([_H, _SK + _S], f16)   # kT | qT
            vT_sb = persist.tile([_H, _SK], f16)
            v_sb = persist.tile([_P, _SKC, _H + 1], bf16)  # v with ones col
            expT = persist.tile([_P, _SKC, _S], bf16)
            oT_sb = persist.tile([_H + 1, _S], f32)

            wu_sb = persist.tile([_P, 512], f16)
            wu_act = persist.tile([_P, 32], bf16)
            nc.vector.memset(wu_sb[:], 0.0)

            nc.vector.memset(v_sb[:, :, _H:_H + 1], 1.0)

            # x DMAs split across the sync and scalar queues -- the one
            # configuration measured to deliver ~2x single-queue aggregate
            # (~370GB/s; gpsimd participation degrades it). The scalar
            # queue's dma_start enqueues retire by ~18us and the exp stream
            # behind them on the in-order ACT queue starts right after,
            # which is the pipeline floor anyway (k+q0 + proj). Priority on
            # both queues: w | k | q0 | q1 | v | q2 | q3.
            nc.sync.dma_start(w_sb[:], w_d.ap())
            nc.scalar.dma_start(bs_sb[:], bs_d.ap())
            nc.scalar.dma_start(xk_sb[:, 4:8], xk_d.ap()[:, 4:8])
            nc.sync.dma_start(xk_sb[:, 0:4], xk_d.ap()[:, 0:4])
            nc.scalar.dma_start(xq_sb[0][:, 4:8], xq_d.ap()[0, :, 4:8])
            nc.sync.dma_start(xq_sb[0][:, 0:4], xq_d.ap()[0, :, 0:4])
            nc.scalar.dma_start(xq_sb[1][:, 4:8], xq_d.ap()[1, :, 4:8])
            nc.sync.dma_start(xq_sb[1][:, 0:4], xq_d.ap()[1, :, 0:4])
            nc.scalar.dma_start(xv_sb[:, 4:8], xv_d.ap()[:, 4:8])
            nc.sync.dma_start(xv_sb[:, 0:4], xv_d.ap()[:, 0:4])
            nc.scalar.dma_start(xq_sb[2][:, 4:8], xq_d.ap()[2, :, 4:8])
            nc.sync.dma_start(xq_sb[2][:, 0:4], xq_d.ap()[2, :, 0:4])
            nc.scalar.dma_start(xq_sb[3][:, 4:8], xq_d.ap()[3, :, 4:8])
            nc.sync.dma_start(xq_sb[3][:, 0:4], xq_d.ap()[3, :, 0:4])
            nc.sync.dma_start(id_sb[:], id_d.ap())

            with tc.tile_pool(name="ps", bufs=1, space="PSUM") as ps_pool:
                # PE warmup burst + keep-warm trickle (HAM clock gate) on
                # memset data; spans the initial x DMA fill
                wu_ps = ps_pool.tile([_P, 512], f32, name="wu", tag="pj",
                                     bufs=2)
                nc.tensor.matmul(wu_ps[:_H, :], wu_sb[:, :_H], wu_sb[:],
                                 start=True, stop=True,
                                 skip_group_check=True)
                for _ in range(_NWU):
                    nc.tensor.matmul(wu_ps[:_H, :_P], wu_sb[:, :_H],
                                     wu_sb[:, :_P], start=True, stop=True,
                                     skip_group_check=True)
                # ACT exp table-set preload (~1.3us), hides under input DMA
                nc.scalar.activation(wu_act[:], wu_sb[:, :32], Exp)

                def proj(widx, rhs_of_e, dst, bias_col):
                    p = ps_pool.tile([_H, 512], f32, name="pj", tag="pj",
                                     bufs=2, padded_shape=[_P, 512])
                    for e in range(_EC):
                        nc.tensor.matmul(
                            p[:], w_sb[:, widx, e, :], rhs_of_e(e),
                            start=(e == 0), stop=(e == _EC - 1),
                        )
                    nc.vector.tensor_scalar_add(dst, p[:], bias_col)

                def scores_exp(j):
                    # two sk-chunks' scores land in one 2-bank PSUM tile so
                    # a single ACTIVATE (the ~270ns/instr overhead engine)
                    # exps 1024 elems/lane at once; dst is a strided
                    # [P, 2, 512] slice of expT
                    rhs = kqT_sb[:, _SK + j * 512:_SK + (j + 1) * 512]
                    for si in range(_SKC // 2):
                        sc = ps_pool.tile([_P, 1024], f32, name="sc", tag="sc",
                                          bufs=2)
                        nc.tensor.matmul(
                            sc[:, 0:512],
                            kqT_sb[:, (2 * si) * _P:(2 * si + 1) * _P],
                            rhs, start=True, stop=True,
                            skip_group_check=True,
                        )
                        nc.tensor.matmul(
                            sc[:, 512:1024],
                            kqT_sb[:, (2 * si + 1) * _P:(2 * si + 2) * _P],
                            rhs, start=True, stop=True,
                            skip_group_check=True,
                        )
                        nc.scalar.activation(
                            expT[:, 2 * si:2 * si + 2, j * 512:(j + 1) * 512],
                            sc[:], Exp)

                def av(j):
                    po = ps_pool.tile([_H + 1, 512], f32, name="po", tag="av",
                                      bufs=2, padded_shape=[_P, 512])
                    for skc in range(_SKC):
                        nc.tensor.matmul(
                            po[:], v_sb[:, skc, :],
                            expT[:, skc, j * 512:(j + 1) * 512],
                            start=(skc == 0), stop=(skc == _SKC - 1),
                        )
                    nc.vector.tensor_copy(oT_sb[:, j * 512:(j + 1) * 512],
                                          po[:])
                    eng = nc.scalar if j == 3 else nc.sync
                    eng.dma_start(out_d.ap()[:, j * 512:(j + 1) * 512],
                                  oT_sb[:, j * 512:(j + 1) * 512])

                bk = bs_sb[:, 0:1]
                bq = bs_sb[:, 1:2]
                bv = bs_sb[:, 2:3]

                # k projections, then q blocks 0/1 with scores+exp chasing
                # the DMA stream
                for kb in range(2):
                    proj(0, lambda e, kb=kb: xk_sb[:, e, kb * 512:(kb + 1) * 512],
                         kqT_sb[:, kb * 512:(kb + 1) * 512], bk)
                for j in (0, 1):
                    proj(1, lambda e, j=j: xq_sb[j][:, e, :],
                         kqT_sb[:, _SK + j * 512:_SK + (j + 1) * 512], bq)
                    scores_exp(j)

                # v projections + PE-transpose rebuild of v (ones col DMA'd)
                for vb in range(2):
                    proj(2, lambda e, vb=vb: xv_sb[:, e, vb * 512:(vb + 1) * 512],
                         vT_sb[:, vb * 512:(vb + 1) * 512], bv)
                for skc in range(_SKC):
                    pvt = ps_pool.tile([_P, _H], f16, name="pvt", tag="sc",
                                       bufs=2, padded_shape=[_P, 1024])
                    nc.tensor.transpose(
                        pvt[:], vT_sb[:, skc * _P:(skc + 1) * _P], id_sb[:])
                    nc.vector.tensor_copy(v_sb[:, skc, :_H], pvt[:])

                av(0)
                av(1)
                for j in (2, 3):
                    proj(1, lambda e, j=j: xq_sb[j][:, e, :],
                         kqT_sb[:, _SK + j * 512:_SK + (j + 1) * 512], bq)
                    scores_exp(j)
                av(2)
                av(3)

    nc.compile()
    return nc


def _prep_core(query, key, value, Wq, bq, Wk, bk, Wv, bv, core):
    b, half = core // 2, core % 2
    xkT = key[b].T[:, half * _SK:(half + 1) * _SK]         # [E, SK]
    xqT = query[b].T                                       # [E, S]
    xvT = value[b].T[:, half * _SK:(half + 1) * _SK]
    xk = xkT.reshape(_EC, _P, _SK).transpose(1, 0, 2)      # [P, EC, SK]
    xq = xqT.reshape(_EC, _P, _NQ, 512).transpose(2, 1, 0, 3)
    xv = xvT.reshape(_EC, _P, _SK).transpose(1, 0, 2)
    w = np.stack(
        [Wk.reshape(_EC, _P, _H), Wq.reshape(_EC, _P, _H),
         Wv.reshape(_EC, _P, _H)], axis=0,
    ).transpose(2, 0, 1, 3)                                # [P, 3, EC, H]
    bs = np.stack(
        [np.asarray(bk, dtype=np.float32).ravel(),
         np.asarray(bq, dtype=np.float32).ravel(),
         np.asarray(bv, dtype=np.float32).ravel()], axis=1,
    )                                                      # [H, 3]
    return {
        "xk": np.ascontiguousarray(xk).astype(np.float16),
        "xq": np.ascontiguousarray(xq).astype(np.float16),
        "xv": np.ascontiguousarray(xv).astype(np.float16),
        "w": np.ascontiguousarray(w).astype(np.float16),
        "bs": np.ascontiguousarray(bs, dtype=np.float32),
        "ident": np.eye(_H, dtype=np.float16),
    }


def _get_built():
    global _built
    if _built is None:
        _built = _build()
    return _built


def kernel(query, key, value, Wq, bq, Wk, bk, Wv, bv, _trace=False):
    from concourse.bass_utils import run_bass_kernel_spmd

    query = np.asarray(query, dtype=np.float32)
    key = np.asarray(key, dtype=np.float32)
    value = np.asarray(value, dtype=np.float32)
    Wq = np.asarray(Wq, dtype=np.float32)
    Wk = np.asarray(Wk, dtype=np.float32)
    Wv = np.asarray(Wv, dtype=np.float32)

    nc = _get_built()
    in_maps = [
        _prep_core(query, key, value, Wq, bq, Wk, bk, Wv, bv, c) for c in range(8)
    ]
    res = run_bass_kernel_spmd(nc, in_maps, core_ids=list(range(8)), trace=_trace)
    out = np.empty((_B, _S, _H), dtype=np.float32)
    for b in range(_B):
        oA = res.results[2 * b]["out"]      # [H+1, S]
        oB = res.results[2 * b + 1]["out"]
        num = oA[: _H] + oB[: _H]
        den = oA[_H] + oB[_H]
        out[b] = (num / den).T
    if _trace:
        kernel.last_result = res
    return out


# revision 16
# speedup vs baseline: 1.2898x; 1.0176x over previous
"""Single-head attention (B=4, S=2048, E=1024, H=64) on 8 TRN2 NeuronCores.

Sharding: each batch b is handled by a core pair; core 2b takes keys/values
[0:1024), core 2b+1 takes [1024:2048). Each core computes, for ALL 2048
queries of its batch, the unnormalized attention numerator and denominator
over its key half; the host sums the two halves and divides.

Precision: matmul inputs are fp16 (1 cycle/row on the PE like bf16, but
8x finer mantissa -- all tensors here are unit-scale Gaussians, well inside
fp16 range). Only the exp outputs (up to e^50) need bf16's exponent range:
expT / v / ones are bf16. PSUM accumulation is fp32 and the final softmax
combine is fp32 on the host.

Measured v2 lessons baked in:
  - DMA packets = per-partition contiguous lines; 1-2KB lines run ~180GB/s
    vs ~310GB/s at 8KB+. Host pre-packs x partition-major so every big DMA
    has 8-16KB lines, and there are only ~14 DMAs total (each dma_start
    costs ~620ns of issue time on its queue engine).
  - The PE HAM clock gate re-throttles after ~3.4us idle; a trickle of
    N=128 dummy matmuls covers the initial DMA fill so the first real
    projection runs at 2.4GHz.
  - ACT does exp ONLY (~690ns per 512-wide chunk incl overhead); the exp
    table set is pre-loaded at t=0 (one-time ~1.3us DMA) under the input
    DMA. All PSUM->SBUF copies go to DVE.

DMA priority order: w | k-x | q-x blocks 0,1 | v-x | q-x blocks 2,3, so
scoresT -> exp -> AV chase the q stream and the tail after the last q byte
is ~proj + 8 exp chunks + AV + out.
"""

import numpy as np

_B, _S, _E, _H = 4, 2048, 1024, 64
_P = 128
_EC = _E // _P          # 8 E-chunks
_SK = _S // 2           # 1024 keys per core
_SKC = _SK // _P        # 8 sk chunks
_NQ = _S // 512         # 4 query 512-blocks
_NWU = 48               # PE keep-warm dummy matmuls

_built = None


def _build():
    import concourse.bacc as bacc
    import concourse.mybir as mybir
    import concourse.tile as tile

    f32 = mybir.dt.float32
    f16 = mybir.dt.float16
    bf16 = mybir.dt.bfloat16
    Exp = mybir.ActivationFunctionType.Exp

    nc = bacc.Bacc("TRN2", target_bir_lowering=False, debug=False,
                   enable_asserts=False, num_devices=8)

    xk_d = nc.dram_tensor("xk", [_P, _EC, _SK], f16, kind="ExternalInput")
    xq_d = nc.dram_tensor("xq", [_NQ, _P, _EC, 512], f16, kind="ExternalInput")
    xv_d = nc.dram_tensor("xv", [_P, _EC, _SK], f16, kind="ExternalInput")
    w_d = nc.dram_tensor("w", [_P, 3, _EC, _H], f16, kind="ExternalInput")
    bs_d = nc.dram_tensor("bs", [_H, 3], f32, kind="ExternalInput")
    id_d = nc.dram_tensor("ident", [_H, _H], f16, kind="ExternalInput")
    out_d = nc.dram_tensor("out", [_H + 1, _S], f32, kind="ExternalOutput")

    with tile.TileContext(nc) as tc:
        with (
            tc.tile_pool(name="persist", bufs=1) as persist,
        ):
            w_sb = persist.tile([_P, 3, _EC, _H], f16)
            bs_sb = persist.tile([_H, 3], f32)
            id_sb = persist.tile([_H, _H], f16)

            xk_sb = persist.tile([_P, _EC, _SK], f16)
            xq_sb = [persist.tile([_P, _EC, 512], f16, name=f"xq{j}")
                     for j in range(_NQ)]
            xv_sb = persist.tile([_P, _EC, _SK], f16)

            kqT_sb = persist.tile([_H, _SK + _S], f16)   # kT | qT
            vT_sb = persist.tile([_H, _SK], f16)
            v_sb = persist.tile([_P, _SKC, _H + 1], bf16)  # v with ones col
            expT = persist.tile([_P, _SKC, _S], bf16)
            oT_sb = persist.tile([_H + 1, _S], f32)

            wu_sb = persist.tile([_P, 512], f16)
            wu_act = persist.tile([_P, 32], bf16)
            nc.vector.memset(wu_sb[:], 0.0)

            nc.vector.memset(v_sb[:, :, _H:_H + 1], 1.0)

            # x DMAs split across the sync and scalar queues -- the one
            # configuration measured to deliver ~2x single-queue aggregate
            # (~370GB/s; gpsimd participation degrades it). The scalar
            # queue's dma_start enqueues retire by ~18us and the exp stream
            # behind them on the in-order ACT queue starts right after,
            # which is the pipeline floor anyway (k+q0 + proj). Priority on
            # both queues: w | k | q0 | q1 | v | q2 | q3.
            nc.sync.dma_start(w_sb[:], w_d.ap())
            nc.scalar.dma_start(bs_sb[:], bs_d.ap())
            nc.scalar.dma_start(xk_sb[:, 4:8], xk_d.ap()[:, 4:8])
            nc.sync.dma_start(xk_sb[:, 0:4], xk_d.ap()[:, 0:4])
            nc.scalar.dma_start(xq_sb[0][:, 4:8], xq_d.ap()[0, :, 4:8])
            nc.scalar.dma_start(id_sb[:], id_d.ap())
            nc.sync.dma_start(xq_sb[0][:, 0:4], xq_d.ap()[0, :, 0:4])
            nc.scalar.dma_start(xq_sb[1][:, 4:8], xq_d.ap()[1, :, 4:8])
            nc.sync.dma_start(xq_sb[1][:, 0:4], xq_d.ap()[1, :, 0:4])
            nc.scalar.dma_start(xv_sb[:, 4:8], xv_d.ap()[:, 4:8])
            nc.sync.dma_start(xv_sb[:, 0:4], xv_d.ap()[:, 0:4])
            nc.scalar.dma_start(xq_sb[2][:, 4:8], xq_d.ap()[2, :, 4:8])
            nc.sync.dma_start(xq_sb[2][:, 0:4], xq_d.ap()[2, :, 0:4])
            nc.scalar.dma_start(xq_sb[3][:, 4:8], xq_d.ap()[3, :, 4:8])
            nc.sync.dma_start(xq_sb[3][:, 0:4], xq_d.ap()[3, :, 0:4])

            with tc.tile_pool(name="ps", bufs=1, space="PSUM") as ps_pool:
                # PE warmup burst + keep-warm trickle (HAM clock gate) on
                # memset data; spans the initial x DMA fill
                wu_ps = ps_pool.tile([_P, 512], f32, name="wu", tag="pj",
                                     bufs=2)
                nc.tensor.matmul(wu_ps[:_H, :], wu_sb[:, :_H], wu_sb[:],
                                 start=True, stop=True,
                                 skip_group_check=True)
                for _ in range(_NWU):
                    nc.tensor.matmul(wu_ps[:_H, :_P], wu_sb[:, :_H],
                                     wu_sb[:, :_P], start=True, stop=True,
                                     skip_group_check=True)
                # ACT exp table-set preload (~1.3us), hides under input DMA
                nc.scalar.activation(wu_act[:], wu_sb[:, :32], Exp)

                def proj(widx, rhs_of_e, dst, bias_col):
                    p = ps_pool.tile([_H, 512], f32, name="pj", tag="pj",
                                     bufs=2, padded_shape=[_P, 512])
                    for e in range(_EC):
                        nc.tensor.matmul(
                            p[:], w_sb[:, widx, e, :], rhs_of_e(e),
                            start=(e == 0), stop=(e == _EC - 1),
                        )
                    nc.vector.tensor_scalar_add(dst, p[:], bias_col)

                def scores_exp(j):
                    # two sk-chunks' scores land in one 2-bank PSUM tile so
                    # a single ACTIVATE (the ~270ns/instr overhead engine)
                    # exps 1024 elems/lane at once; dst is a strided
                    # [P, 2, 512] slice of expT
                    rhs = kqT_sb[:, _SK + j * 512:_SK + (j + 1) * 512]
                    for si in range(_SKC // 2):
                        sc = ps_pool.tile([_P, 1024], f32, name="sc", tag="sc",
                                          bufs=2)
                        nc.tensor.matmul(
                            sc[:, 0:512],
                            kqT_sb[:, (2 * si) * _P:(2 * si + 1) * _P],
                            rhs, start=True, stop=True,
                            skip_group_check=True,
                        )
                        nc.tensor.matmul(
                            sc[:, 512:1024],
                            kqT_sb[:, (2 * si + 1) * _P:(2 * si + 2) * _P],
                            rhs, start=True, stop=True,
                            skip_group_check=True,
                        )
                        nc.scalar.activation(
                            expT[:, 2 * si:2 * si + 2, j * 512:(j + 1) * 512],
                            sc[:], Exp)

                def av(j):
                    po = ps_pool.tile([_H + 1, 512], f32, name="po", tag="av",
                                      bufs=2, padded_shape=[_P, 512])
                    for skc in range(_SKC):
                        nc.tensor.matmul(
                            po[:], v_sb[:, skc, :],
                            expT[:, skc, j * 512:(j + 1) * 512],
                            start=(skc == 0), stop=(skc == _SKC - 1),
                        )
                    nc.vector.tensor_copy(oT_sb[:, j * 512:(j + 1) * 512],
                                          po[:])
                    eng = nc.scalar if j == 3 else nc.sync
                    eng.dma_start(out_d.ap()[:, j * 512:(j + 1) * 512],
                                  oT_sb[:, j * 512:(j + 1) * 512])

                bk = bs_sb[:, 0:1]
                bq = bs_sb[:, 1:2]
                bv = bs_sb[:, 2:3]

                # k projections, then q blocks 0/1 with scores+exp chasing
                # the DMA stream
                for kb in range(2):
                    proj(0, lambda e, kb=kb: xk_sb[:, e, kb * 512:(kb + 1) * 512],
                         kqT_sb[:, kb * 512:(kb + 1) * 512], bk)
                for j in (0, 1):
                    proj(1, lambda e, j=j: xq_sb[j][:, e, :],
                         kqT_sb[:, _SK + j * 512:_SK + (j + 1) * 512], bq)
                    scores_exp(j)

                # v projections + PE-transpose rebuild of v (ones col DMA'd)
                for vb in range(2):
                    proj(2, lambda e, vb=vb: xv_sb[:, e, vb * 512:(vb + 1) * 512],
                         vT_sb[:, vb * 512:(vb + 1) * 512], bv)
                for skc in range(_SKC):
                    pvt = ps_pool.tile([_P, _H], f16, name="pvt", tag="sc",
                                       bufs=2, padded_shape=[_P, 1024])
                    nc.tensor.transpose(
                        pvt[:], vT_sb[:, skc * _P:(skc + 1) * _P], id_sb[:])
                    nc.vector.tensor_copy(v_sb[:, skc, :_H], pvt[:])

                av(0)
                av(1)
                for j in (2, 3):
                    proj(1, lambda e, j=j: xq_sb[j][:, e, :],
                         kqT_sb[:, _SK + j * 512:_SK + (j + 1) * 512], bq)
                    scores_exp(j)
                av(2)
                av(3)

    nc.compile()
    return nc


def _prep_core(query, key, value, Wq, bq, Wk, bk, Wv, bv, core):
    b, half = core // 2, core % 2
    xkT = key[b].T[:, half * _SK:(half + 1) * _SK]         # [E, SK]
    xqT = query[b].T                                       # [E, S]
    xvT = value[b].T[:, half * _SK:(half + 1) * _SK]
    xk = xkT.reshape(_EC, _P, _SK).transpose(1, 0, 2)      # [P, EC, SK]
    xq = xqT.reshape(_EC, _P, _NQ, 512).transpose(2, 1, 0, 3)
    xv = xvT.reshape(_EC, _P, _SK).transpose(1, 0, 2)
    w = np.stack(
        [Wk.reshape(_EC, _P, _H), Wq.reshape(_EC, _P, _H),
         Wv.reshape(_EC, _P, _H)], axis=0,
    ).transpose(2, 0, 1, 3)                                # [P, 3, EC, H]
    bs = np.stack(
        [np.asarray(bk, dtype=np.float32).ravel(),
         np.asarray(bq, dtype=np.float32).ravel(),
         np.asarray(bv, dtype=np.float32).ravel()], axis=1,
    )                                                      # [H, 3]
    return {
        "xk": np.ascontiguousarray(xk).astype(np.float16),
        "xq": np.ascontiguousarray(xq).astype(np.float16),
        "xv": np.ascontiguousarray(xv).astype(np.float16),
        "w": np.ascontiguousarray(w).astype(np.float16),
        "bs": np.ascontiguousarray(bs, dtype=np.float32),
        "ident": np.eye(_H, dtype=np.float16),
    }


def _get_built():
    global _built
    if _built is None:
        _built = _build()
    return _built


def kernel(query, key, value, Wq, bq, Wk, bk, Wv, bv, _trace=False):
    from concourse.bass_utils import run_bass_kernel_spmd

    query = np.asarray(query, dtype=np.float32)
    key = np.asarray(key, dtype=np.float32)
    value = np.asarray(value, dtype=np.float32)
    Wq = np.asarray(Wq, dtype=np.float32)
    Wk = np.asarray(Wk, dtype=np.float32)
    Wv = np.asarray(Wv, dtype=np.float32)

    nc = _get_built()
    in_maps = [
        _prep_core(query, key, value, Wq, bq, Wk, bk, Wv, bv, c) for c in range(8)
    ]
    res = run_bass_kernel_spmd(nc, in_maps, core_ids=list(range(8)), trace=_trace)
    out = np.empty((_B, _S, _H), dtype=np.float32)
    for b in range(_B):
        oA = res.results[2 * b]["out"]      # [H+1, S]
        oB = res.results[2 * b + 1]["out"]
        num = oA[: _H] + oB[: _H]
        den = oA[_H] + oB[_H]
        out[b] = (num / den).T
    if _trace:
        kernel.last_result = res
    return out
